# revision 4
# baseline (speedup 1.0000x reference)
"""Trainium2 Bass kernel for a 3-layer bidirectional projected-LSTM embedder.

Model (from the reference):
  T=160, B=640, F=40, HID=768, PROJ=256, 3 stacked LSTM-with-projection
  layers per direction (fw, bw).  Per step:
      z = [x_t, h_{t-1}] @ Wk + b            # [B, 4*HID], gate order i,j,f,o
      c = sig(f+1)*c + sig(i)*tanh(j)
      h = (sig(o)*tanh(c)) @ Wp              # [B, PROJ]
  Output = l2norm((concat(fw,bw)[t=0] + concat(fw,bw)[t=T-1]) / 2)  # [B, 512]

Strategy: pure data-parallel over batch (80 per core, 8 cores, no
collectives).  Per core the three layers run as sequential phases; within a
phase the fw and bw recurrences are interleaved so PE/ACT/DVE overlap.  The
whole z path is bf16 (weights, x, h) with fp32 PSUM accumulation -- simulated
end-to-end rel-err 1.1e-2 vs the 2e-2 budget.  z = lhsT.T @ Wk with the
activations as the stationary operand and the (SBUF-resident, double-buffered
across layer phases) weights streaming at 1 col/cycle.  s^T and h^T are
produced by DMA-engine xbar transposes (SBUF->SBUF, off the PE).  Layer-to-
layer h sequences ping-pong through DRAM in bf16.  The final (t0+tT)/2 +
l2-normalize runs on the host in numpy.
"""

import numpy as np

T, B, F = 160, 640, 40
HID, PROJ = 768, 256
NG = 4 * HID          # 3072
NCORES = 8
BC = B // NCORES      # 80
NKH = PROJ // 128     # 2 k-tiles for the recurrent part

_BUILD_CACHE = {}



def _build(t_steps, cw=512, dma_t=True):
    from contextlib import ExitStack

    import concourse.bass as bass  # noqa: F401
    import concourse.tile as tile
    from concourse import bacc, mybir
    from concourse.masks import make_identity

    f32 = mybir.dt.float32
    bf16 = mybir.dt.bfloat16
    AF = mybir.ActivationFunctionType

    DIRS = ("fw", "bw")
    CW = cw

    nc = bacc.Bacc(None, target_bir_lowering=False)

    xT = nc.declare_dram_parameter("xT", [F, t_steps * BC], bf16, isOutput=False)
    wk_in = {}
    wp_in = {}
    for d in DIRS:
        for l in range(3):
            # uniform padded layout [512, NG]; L0: rows 0:40 = x-part,
            # rows 128:384 = h-part, rest zero (tile 3 never streamed)
            wk_in[d, l] = nc.declare_dram_parameter(
                f"Wk_{d}{l}", [512, NG], bf16, isOutput=False)
            wp_in[d, l] = nc.declare_dram_parameter(
                f"Wp_{d}{l}", [HID, PROJ], bf16, isOutput=False)
    # hT of the top layer at t=0 and t=T-1:  [dir, end, 128, kt, BC]
    out_ends = nc.declare_dram_parameter(
        "out_ends", [2, 2, 128, NKH, BC], bf16, isOutput=True)

    with tile.TileContext(nc) as tc:
        with ExitStack() as top:
            dram = top.enter_context(tc.tile_pool(name="dram", bufs=1, space="DRAM"))
            # weights double-buffered across layer phases (prefetch l+1
            # while computing l)
            wpool = top.enter_context(tc.tile_pool(name="w", bufs=2))
            if not dma_t:
                glob = top.enter_context(tc.tile_pool(name="glob", bufs=1))
                ident_bf = glob.tile([BC, BC], bf16)
                make_identity(nc, ident_bf)

            # layer-to-layer h^T sequences (ping-pong per direction)
            hseq = {}
            for d in DIRS:
                for i in (0, 1):
                    hseq[d, i] = dram.tile([128, NKH, t_steps, BC], bf16,
                                           name=f"hseq_{d}{i}", tag=f"hseq_{d}{i}")

            def load_weights(l):
                # k-tiles actually streamed this layer: L0 -> 3, else 4
                wk_t = {d: [] for d in DIRS}
                wp_t = {d: [] for d in DIRS}
                for d in DIRS:
                    for ki in range(4):
                        wt = wpool.tile([128, NG], bf16,
                                        name=f"wk_{d}{l}_{ki}",
                                        tag=f"wk_{d}_{ki}")
                        if not (l == 0 and ki == 3):
                            for c in range(3):
                                nc.sync.dma_start(
                                    out=wt[:, c * 1024:(c + 1) * 1024],
                                    in_=wk_in[d, l][ki * 128:(ki + 1) * 128,
                                                    c * 1024:(c + 1) * 1024])
                        wk_t[d].append(wt)
                    for ki in range(6):
                        pt = wpool.tile([128, PROJ], bf16,
                                        name=f"wp_{d}{l}_{ki}",
                                        tag=f"wp_{d}_{ki}")
                        nc.sync.dma_start(
                            out=pt, in_=wp_in[d, l][ki * 128:(ki + 1) * 128, :])
                        wp_t[d].append(pt)
                return wk_t, wp_t

            for l in range(3):
                with ExitStack() as ph:
                    spool = ph.enter_context(tc.tile_pool(name=f"s{l}", bufs=1))
                    gpool = ph.enter_context(tc.tile_pool(name=f"g{l}", bufs=1))
                    xpool = ph.enter_context(tc.tile_pool(name=f"x{l}", bufs=6))
                    zpool = ph.enter_context(
                        tc.tile_pool(name=f"z{l}", bufs=1, space="PSUM"))
                    apool = ph.enter_context(
                        tc.tile_pool(name=f"a{l}", bufs=1, space="PSUM"))

                    wk_t, wp_t = load_weights(l)

                    # ---- state ----
                    st = {}
                    for d in DIRS:
                        c_sb = spool.tile([BC, HID], f32, name=f"c_{d}{l}",
                                          tag=f"c_{d}")
                        st[d] = [c_sb, None]   # hT produced by step 0

                    for step in range(t_steps):
                        for d in DIRS:
                            t = step if d == "fw" else t_steps - 1 - step
                            c_sb, hT = st[d]

                            if l == 0:
                                xin0 = xpool.tile([F, BC], bf16,
                                                  name=f"xin0_{d}", tag=f"xin_{d}")
                                nc.sync.dma_start(
                                    out=xin0, in_=xT[:, t * BC:(t + 1) * BC])
                                # (lhsT, wk_tile_idx, k_rows)
                                xparts = [(xin0, 0, F)]
                            else:
                                xin = xpool.tile([128, NKH * BC], bf16,
                                                 name=f"xin_{d}{l}",
                                                 tag=f"xin_{d}")
                                nc.sync.dma_start(
                                    out=xin.rearrange("p (k b) -> p k b", k=NKH),
                                    in_=hseq[d, (l - 1) % 2][:, :, t, :])
                                xparts = [(xin[:, ki * BC:(ki + 1) * BC], ki, 128)
                                          for ki in range(NKH)]
                            hki0 = 1 if l == 0 else 2
                            if step == 0:
                                lhsts = xparts   # h_{-1} = 0
                            else:
                                lhsts = xparts + [
                                    (hT[:, ki * BC:(ki + 1) * BC], hki0 + ki, 128)
                                    for ki in range(NKH)]

                            # z = [x, h] @ Wk  -> chunks of [BC, CW] in PSUM
                            nch = NG // CW
                            zc = []
                            for c in range(nch):
                                zt = zpool.tile([BC, CW], f32,
                                                name=f"z{c}_{d}{l}", tag=f"z{c}")
                                for ns in range(CW // 512):
                                    cols = slice(c * CW + ns * 512,
                                                 c * CW + (ns + 1) * 512)
                                    for li, (lt, wki, krows) in enumerate(lhsts):
                                        nc.tensor.matmul(
                                            zt[:, ns * 512:(ns + 1) * 512],
                                            lt, wk_t[d][wki][0:krows, cols],
                                            start=(li == 0),
                                            stop=(li == len(lhsts) - 1))
                                zc.append(zt)

                            # gates (gate g spans z cols [g*HID, (g+1)*HID))
                            gt = {}
                            for g, fn, bias in ((0, AF.Sigmoid, 0.0),
                                                (1, AF.Tanh, 0.0),
                                                (2, AF.Sigmoid, 1.0),
                                                (3, AF.Sigmoid, 0.0)):
                                gt[g] = gpool.tile([BC, HID], f32,
                                                   name=f"g{g}_{d}{l}",
                                                   tag=f"g{g}_{d}")
                                glo, ghi = g * HID, (g + 1) * HID
                                for c in range(glo // CW, (ghi - 1) // CW + 1):
                                    lo, hi = max(glo, c * CW), min(ghi, (c + 1) * CW)
                                    nc.scalar.activation(
                                        gt[g][:, lo - glo:hi - glo],
                                        zc[c][:, lo - c * CW:hi - c * CW],
                                        fn, bias=bias)

                            # c = sig(f+1)*c + sig(i)*tanh(j)
                            if step == 0:
                                nc.vector.tensor_mul(c_sb, gt[0], gt[1])
                            else:
                                tmp = gpool.tile([BC, HID], f32,
                                                 name=f"tmp_{d}{l}", tag=f"tmp_{d}")
                                nc.vector.tensor_mul(tmp, gt[0], gt[1])
                                nc.vector.tensor_mul(c_sb, gt[2], c_sb)
                                nc.vector.tensor_add(c_sb, c_sb, tmp)
                            tanhc = gpool.tile([BC, HID], f32,
                                               name=f"tanhc_{d}{l}",
                                               tag=f"tanhc_{d}")
                            nc.scalar.activation(tanhc, c_sb, AF.Tanh)
                            s_sb = gpool.tile([BC, HID], bf16,
                                              name=f"s_{d}{l}", tag=f"s_{d}")
                            nc.vector.tensor_mul(s_sb, gt[3], tanhc)

                            # s^T [768(6x128), BC] via DMA xbar transpose
                            sT_sb = gpool.tile([128, 6 * BC], bf16,
                                               name=f"sT_{d}{l}", tag=f"sT_{d}")
                            if dma_t:
                                for j in range(6):
                                    nc.sync.dma_start(
                                        out=sT_sb[:, j * BC:(j + 1) * BC],
                                        in_=s_sb[:, j * 128:(j + 1) * 128],
                                        transpose=True)
                            else:
                                sT_ps = apool.tile([128, 6 * BC], bf16,
                                                   name=f"sTp_{d}{l}", tag="sT")
                                for j in range(6):
                                    nc.tensor.transpose(
                                        sT_ps[:, j * BC:(j + 1) * BC],
                                        s_sb[:, j * 128:(j + 1) * 128], ident_bf)
                                nc.vector.tensor_copy(sT_sb, sT_ps)

                            # h = s @ Wp  [BC, PROJ] (fp32 PSUM) -> bf16 SBUF
                            h_ps = apool.tile([BC, PROJ], f32,
                                              name=f"hp_{d}{l}", tag="hps")
                            for ki in range(6):
                                nc.tensor.matmul(
                                    h_ps, sT_sb[:, ki * BC:(ki + 1) * BC],
                                    wp_t[d][ki], start=(ki == 0), stop=(ki == 5))
                            h_sb = gpool.tile([BC, PROJ], bf16,
                                              name=f"h_{d}{l}", tag=f"h_{d}")
                            nc.vector.tensor_copy(h_sb, h_ps)
                            # h^T [256(2x128), BC] via DMA xbar transpose
                            hT_new = spool.tile([128, NKH * BC], bf16,
                                                name=f"hTn_{d}{l}", tag=f"hT_{d}")
                            if dma_t:
                                for j in range(NKH):
                                    nc.sync.dma_start(
                                        out=hT_new[:, j * BC:(j + 1) * BC],
                                        in_=h_sb[:, j * 128:(j + 1) * 128],
                                        transpose=True)
                            else:
                                hT_ps = apool.tile([128, NKH * BC], bf16,
                                                   name=f"hTp_{d}{l}", tag="hTp")
                                for j in range(NKH):
                                    nc.tensor.transpose(
                                        hT_ps[:, j * BC:(j + 1) * BC],
                                        h_sb[:, j * 128:(j + 1) * 128], ident_bf)
                                nc.vector.tensor_copy(hT_new, hT_ps)
                            st[d][1] = hT_new

                            if l < 2:
                                nc.sync.dma_start(
                                    out=hseq[d, l % 2][:, :, t, :],
                                    in_=hT_new.rearrange("p (k b) -> p k b", k=NKH))
                            else:
                                di = 0 if d == "fw" else 1
                                if t == 0:
                                    nc.sync.dma_start(
                                        out=out_ends[di, 0],
                                        in_=hT_new.rearrange("p (k b) -> p k b",
                                                             k=NKH))
                                if t == t_steps - 1:
                                    nc.sync.dma_start(
                                        out=out_ends[di, 1],
                                        in_=hT_new.rearrange("p (k b) -> p k b",
                                                             k=NKH))

    nc.finalize()
    return nc


def _get_nc(t_steps=T, cw=512, dma_t=True):
    key = (t_steps, cw, dma_t)
    if key not in _BUILD_CACHE:
        _BUILD_CACHE[key] = _build(t_steps, cw, dma_t)
    return _BUILD_CACHE[key]


def _make_in_maps(inputs):
    """Pack full inputs into per-core in_maps (bf16, padded Wk layout)."""
    import ml_dtypes
    bf = ml_dtypes.bfloat16

    inp = {k: np.asarray(v, dtype=np.float32) for k, v in inputs.items()}
    batch = inp["batch"]
    assert batch.shape == (T, B, F), batch.shape

    shared = {}
    for d in ("fw", "bw"):
        for l in range(3):
            wk = inp[f"Wk_{d}{l}"]          # TF gate order i,j,f,o (matches
            b = inp[f"b_{d}{l}"]            # the kernel's gate loop directly)
            assert not np.any(b), "bias path removed (reference uses b=0)"
            ind = wk.shape[0] - PROJ
            pk = np.zeros((512, NG), dtype=np.float32)
            pk[0:ind] = wk[0:ind]                       # x-part
            hk0 = 128 if l == 0 else ind
            pk[hk0:hk0 + PROJ] = wk[ind:]               # h-part at k-tile 1 or 2
            shared[f"Wk_{d}{l}"] = np.ascontiguousarray(pk.astype(bf))
            shared[f"Wp_{d}{l}"] = np.ascontiguousarray(
                inp[f"Wp_{d}{l}"].astype(bf))

    in_maps = []
    for i in range(NCORES):
        xb = batch[:, i * BC:(i + 1) * BC, :]           # [T, BC, F]
        xT_i = np.ascontiguousarray(
            xb.transpose(2, 0, 1).reshape(F, T * BC).astype(bf))  # [F, T*BC]
        in_maps.append({"xT": xT_i, **shared})
    return in_maps


def kernel(**inputs):
    from concourse.bass_utils import run_bass_kernel_spmd

    nc = _get_nc(T)
    in_maps = _make_in_maps(inputs)
    res = run_bass_kernel_spmd(nc, in_maps, core_ids=list(range(NCORES)))

    # assemble: out_ends [2(dir), 2(end), 128, NKH, BC] -> h [BC, 256]
    h = np.zeros((2, 2, B, PROJ), dtype=np.float32)    # [dir, end, B, PROJ]
    for i in range(NCORES):
        oe = res.results[i]["out_ends"].astype(np.float32)
        # h[b, kt*128 + p] = oe[.., p, kt, b]
        h[:, :, i * BC:(i + 1) * BC, :] = oe.transpose(0, 1, 4, 3, 2).reshape(
            2, 2, BC, PROJ)

    out0 = np.concatenate([h[0, 0], h[1, 0]], axis=1)   # t = 0
    outT = np.concatenate([h[0, 1], h[1, 1]], axis=1)   # t = T-1
    emb = (out0 + outT) / np.float32(2.0)
    ss = np.maximum(np.sum(emb * emb, axis=-1, keepdims=True), np.float32(1e-12))
    emb = emb / np.sqrt(ss)
    return emb.astype(np.float32)


# revision 10
# speedup vs baseline: 2.5623x; 2.5623x over previous
"""Trainium2 Bass kernel for a 3-layer bidirectional projected-LSTM embedder.

Model (from the reference):
  T=160, B=640, F=40, HID=768, PROJ=256, 3 stacked LSTM-with-projection
  layers per direction (fw, bw).  Per step:
      z = [x_t, h_{t-1}] @ Wk + b            # [B, 4*HID], gate order i,j,f,o
      c = sig(f+1)*c + sig(i)*tanh(j)
      h = (sig(o)*tanh(c)) @ Wp              # [B, PROJ]
  Output = l2norm((concat(fw,bw)[t=0] + concat(fw,bw)[t=T-1]) / 2)  # [B, 512]

Strategy: pure data-parallel over batch (80 per core, 8 cores, no
collectives).  Per core the three layers run as sequential phases; within a
phase the fw and bw recurrences are interleaved so PE/ACT/DVE overlap.  The
whole z path is bf16 (weights, x, h) with fp32 PSUM accumulation -- simulated
end-to-end rel-err 1.1e-2 vs the 2e-2 budget.  z = lhsT.T @ Wk with the
activations as the stationary operand and the (SBUF-resident, double-buffered
across layer phases) weights streaming at 1 col/cycle.  s^T and h^T are
produced by DMA-engine xbar transposes (SBUF->SBUF, off the PE).  Layer-to-
layer h sequences ping-pong through DRAM in bf16.  The final (t0+tT)/2 +
l2-normalize runs on the host in numpy.
"""

import numpy as np

T, B, F = 160, 640, 40
HID, PROJ = 768, 256
NG = 4 * HID          # 3072
NCORES = 8
BC = B // NCORES      # 80
NKH = PROJ // 128     # 2 k-tiles for the recurrent part

_BUILD_CACHE = {}



def _build(t_steps, cw=512, dma_t=False):
    from contextlib import ExitStack

    import concourse.bass as bass  # noqa: F401
    import concourse.tile as tile
    from concourse import bacc, mybir
    from concourse.masks import make_identity

    f32 = mybir.dt.float32
    bf16 = mybir.dt.bfloat16
    AF = mybir.ActivationFunctionType

    DIRS = ("fw", "bw")
    CW = cw

    nc = bacc.Bacc(None, target_bir_lowering=False)

    xT = nc.declare_dram_parameter("xT", [F, t_steps * BC], bf16, isOutput=False)
    wk_in = {}
    wp_in = {}
    for d in DIRS:
        for l in range(3):
            # uniform padded layout [512, NG]; L0: rows 0:40 = x-part,
            # rows 128:384 = h-part, rest zero (tile 3 never streamed)
            wk_in[d, l] = nc.declare_dram_parameter(
                f"Wk_{d}{l}", [512, NG], bf16, isOutput=False)
            wp_in[d, l] = nc.declare_dram_parameter(
                f"Wp_{d}{l}", [HID, PROJ], bf16, isOutput=False)
    # hT of the top layer at t=0 and t=T-1:  [dir, end, 128, kt, BC]
    out_ends = nc.declare_dram_parameter(
        "out_ends", [2, 2, 128, NKH, BC], bf16, isOutput=True)

    with tile.TileContext(nc) as tc:
        with ExitStack() as top:
            dram = top.enter_context(tc.tile_pool(name="dram", bufs=1, space="DRAM"))
            # weights double-buffered across layer phases (prefetch l+1
            # while computing l)
            wpool = top.enter_context(tc.tile_pool(name="w", bufs=2))
            glob = top.enter_context(tc.tile_pool(name="glob", bufs=1))
            if not dma_t:
                ident_bf = glob.tile([BC, BC], bf16)
                make_identity(nc, ident_bf)
            # L0 input resident in SBUF: [F, T*BC] bf16 = 1 MB
            xT_sb = glob.tile([F, t_steps * BC], bf16, name="xT_sb")
            for c in range(8):
                cs = t_steps * BC // 8
                nc.sync.dma_start(out=xT_sb[:, c * cs:(c + 1) * cs],
                                  in_=xT[:, c * cs:(c + 1) * cs])

            # layer-to-layer h^T sequences (ping-pong per direction)
            hseq = {}
            for d in DIRS:
                for i in (0, 1):
                    hseq[d, i] = dram.tile([128, NKH, t_steps, BC], bf16,
                                           name=f"hseq_{d}{i}", tag=f"hseq_{d}{i}")

            def load_weights(l):
                # k-tiles actually streamed this layer: L0 -> 3, else 4
                wk_t = {d: [] for d in DIRS}
                wp_t = {d: [] for d in DIRS}
                for d in DIRS:
                    for ki in range(4):
                        wt = wpool.tile([128, NG], bf16,
                                        name=f"wk_{d}{l}_{ki}",
                                        tag=f"wk_{d}_{ki}")
                        if not (l == 0 and ki == 3):
                            for c in range(3):
                                nc.sync.dma_start(
                                    out=wt[:, c * 1024:(c + 1) * 1024],
                                    in_=wk_in[d, l][ki * 128:(ki + 1) * 128,
                                                    c * 1024:(c + 1) * 1024])
                        wk_t[d].append(wt)
                    for ki in range(6):
                        pt = wpool.tile([128, PROJ], bf16,
                                        name=f"wp_{d}{l}_{ki}",
                                        tag=f"wp_{d}_{ki}")
                        nc.sync.dma_start(
                            out=pt, in_=wp_in[d, l][ki * 128:(ki + 1) * 128, :])
                        wp_t[d].append(pt)
                return wk_t, wp_t

            for l in range(3):
                with ExitStack() as ph:
                    spool = ph.enter_context(tc.tile_pool(name=f"s{l}", bufs=1))
                    gpool = ph.enter_context(tc.tile_pool(name=f"g{l}", bufs=1))
                    xpool = ph.enter_context(tc.tile_pool(name=f"x{l}", bufs=6))
                    zpool = ph.enter_context(
                        tc.tile_pool(name=f"z{l}", bufs=1, space="PSUM"))
                    apool = ph.enter_context(
                        tc.tile_pool(name=f"a{l}", bufs=1, space="PSUM"))

                    wk_t, wp_t = load_weights(l)

                    # ---- state ----
                    st = {}
                    for d in DIRS:
                        c_sb = spool.tile([BC, HID], f32, name=f"c_{d}{l}",
                                          tag=f"c_{d}")
                        st[d] = [c_sb, None]   # hT produced by step 0

                    for step in range(t_steps):
                        for d in DIRS:
                            t = step if d == "fw" else t_steps - 1 - step
                            c_sb, hT = st[d]

                            if l == 0:
                                # (lhsT, wk_tile_idx, k_rows)
                                xparts = [(xT_sb[:, t * BC:(t + 1) * BC], 0, F)]
                            else:
                                xin = xpool.tile([128, NKH * BC], bf16,
                                                 name=f"xin_{d}{l}",
                                                 tag=f"xin_{d}")
                                nc.sync.dma_start(
                                    out=xin.rearrange("p (k b) -> p k b", k=NKH),
                                    in_=hseq[d, (l - 1) % 2][:, :, t, :])
                                xparts = [(xin[:, ki * BC:(ki + 1) * BC], ki, 128)
                                          for ki in range(NKH)]
                            hki0 = 1 if l == 0 else 2
                            if step == 0:
                                lhsts = xparts   # h_{-1} = 0
                            else:
                                lhsts = xparts + [
                                    (hT[:, ki * BC:(ki + 1) * BC], hki0 + ki, 128)
                                    for ki in range(NKH)]

                            # z = [x, h] @ Wk  -> chunks of [BC, CW] in PSUM
                            nch = NG // CW
                            zc = []
                            for c in range(nch):
                                zt = zpool.tile([BC, CW], f32,
                                                name=f"z{c}_{d}{l}", tag=f"z{c}")
                                for ns in range(CW // 512):
                                    cols = slice(c * CW + ns * 512,
                                                 c * CW + (ns + 1) * 512)
                                    for li, (lt, wki, krows) in enumerate(lhsts):
                                        nc.tensor.matmul(
                                            zt[:, ns * 512:(ns + 1) * 512],
                                            lt, wk_t[d][wki][0:krows, cols],
                                            start=(li == 0),
                                            stop=(li == len(lhsts) - 1))
                                zc.append(zt)

                            # gates (gate g spans z cols [g*HID, (g+1)*HID))
                            gt = {}
                            for g, fn, bias in ((0, AF.Sigmoid, 0.0),
                                                (1, AF.Tanh, 0.0),
                                                (2, AF.Sigmoid, 1.0),
                                                (3, AF.Sigmoid, 0.0)):
                                gt[g] = gpool.tile([BC, HID], f32,
                                                   name=f"g{g}_{d}{l}",
                                                   tag=f"g{g}_{d}")
                                glo, ghi = g * HID, (g + 1) * HID
                                for c in range(glo // CW, (ghi - 1) // CW + 1):
                                    lo, hi = max(glo, c * CW), min(ghi, (c + 1) * CW)
                                    nc.scalar.activation(
                                        gt[g][:, lo - glo:hi - glo],
                                        zc[c][:, lo - c * CW:hi - c * CW],
                                        fn, bias=bias)

                            # c = sig(f+1)*c + sig(i)*tanh(j)
                            if step == 0:
                                nc.vector.tensor_mul(c_sb, gt[0], gt[1])
                            else:
                                tmp = gpool.tile([BC, HID], f32,
                                                 name=f"tmp_{d}{l}", tag=f"tmp_{d}")
                                nc.vector.tensor_mul(tmp, gt[0], gt[1])
                                nc.vector.tensor_mul(c_sb, gt[2], c_sb)
                                nc.vector.tensor_add(c_sb, c_sb, tmp)
                            tanhc = gpool.tile([BC, HID], f32,
                                               name=f"tanhc_{d}{l}",
                                               tag=f"tanhc_{d}")
                            nc.scalar.activation(tanhc, c_sb, AF.Tanh)
                            s_sb = gpool.tile([BC, HID], bf16,
                                              name=f"s_{d}{l}", tag=f"s_{d}")
                            nc.vector.tensor_mul(s_sb, gt[3], tanhc)

                            # s^T [768(6x128), BC] via DMA xbar transpose
                            sT_sb = gpool.tile([128, 6 * BC], bf16,
                                               name=f"sT_{d}{l}", tag=f"sT_{d}")
                            if dma_t:
                                for j in range(6):
                                    nc.sync.dma_start(
                                        out=sT_sb[:, j * BC:(j + 1) * BC],
                                        in_=s_sb[:, j * 128:(j + 1) * 128],
                                        transpose=True)
                            else:
                                # one PSUM bank shared by s^T and h^T outputs
                                tp_ps = apool.tile([128, 8 * BC], bf16,
                                                   name=f"tp_{d}{l}", tag="tp")
                                sT_ps = tp_ps[:, 0:6 * BC]
                                for j in range(6):
                                    nc.tensor.transpose(
                                        sT_ps[:, j * BC:(j + 1) * BC],
                                        s_sb[:, j * 128:(j + 1) * 128], ident_bf)
                                nc.vector.tensor_copy(sT_sb, sT_ps)

                            # h = s @ Wp  [BC, PROJ] (fp32 PSUM) -> bf16 SBUF
                            h_ps = apool.tile([BC, PROJ], f32,
                                              name=f"hp_{d}{l}", tag="hps")
                            for ki in range(6):
                                nc.tensor.matmul(
                                    h_ps, sT_sb[:, ki * BC:(ki + 1) * BC],
                                    wp_t[d][ki], start=(ki == 0), stop=(ki == 5))
                            h_sb = gpool.tile([BC, PROJ], bf16,
                                              name=f"h_{d}{l}", tag=f"h_{d}")
                            nc.vector.tensor_copy(h_sb, h_ps)
                            # h^T [256(2x128), BC] via DMA xbar transpose
                            hT_new = spool.tile([128, NKH * BC], bf16,
                                                name=f"hTn_{d}{l}", tag=f"hT_{d}")
                            if dma_t:
                                for j in range(NKH):
                                    nc.sync.dma_start(
                                        out=hT_new[:, j * BC:(j + 1) * BC],
                                        in_=h_sb[:, j * 128:(j + 1) * 128],
                                        transpose=True)
                            else:
                                hT_ps = tp_ps[:, 6 * BC:(6 + NKH) * BC]
                                for j in range(NKH):
                                    nc.tensor.transpose(
                                        hT_ps[:, j * BC:(j + 1) * BC],
                                        h_sb[:, j * 128:(j + 1) * 128], ident_bf)
                                nc.vector.tensor_copy(hT_new, hT_ps)
                            st[d][1] = hT_new

                            if l < 2:
                                nc.sync.dma_start(
                                    out=hseq[d, l % 2][:, :, t, :],
                                    in_=hT_new.rearrange("p (k b) -> p k b", k=NKH))
                            else:
                                di = 0 if d == "fw" else 1
                                if t == 0:
                                    nc.sync.dma_start(
                                        out=out_ends[di, 0],
                                        in_=hT_new.rearrange("p (k b) -> p k b",
                                                             k=NKH))
                                if t == t_steps - 1:
                                    nc.sync.dma_start(
                                        out=out_ends[di, 1],
                                        in_=hT_new.rearrange("p (k b) -> p k b",
                                                             k=NKH))

    nc.finalize()
    return nc


def _get_nc(t_steps=T, cw=512, dma_t=False):
    key = (t_steps, cw, dma_t)
    if key not in _BUILD_CACHE:
        _BUILD_CACHE[key] = _build(t_steps, cw, dma_t)
    return _BUILD_CACHE[key]


def _make_in_maps(inputs):
    """Pack full inputs into per-core in_maps (bf16, padded Wk layout)."""
    import ml_dtypes
    bf = ml_dtypes.bfloat16

    inp = {k: np.asarray(v, dtype=np.float32) for k, v in inputs.items()}
    batch = inp["batch"]
    assert batch.shape == (T, B, F), batch.shape

    shared = {}
    for d in ("fw", "bw"):
        for l in range(3):
            wk = inp[f"Wk_{d}{l}"]          # TF gate order i,j,f,o (matches
            b = inp[f"b_{d}{l}"]            # the kernel's gate loop directly)
            assert not np.any(b), "bias path removed (reference uses b=0)"
            ind = wk.shape[0] - PROJ
            pk = np.zeros((512, NG), dtype=np.float32)
            pk[0:ind] = wk[0:ind]                       # x-part
            hk0 = 128 if l == 0 else ind
            pk[hk0:hk0 + PROJ] = wk[ind:]               # h-part at k-tile 1 or 2
            shared[f"Wk_{d}{l}"] = np.ascontiguousarray(pk.astype(bf))
            shared[f"Wp_{d}{l}"] = np.ascontiguousarray(
                inp[f"Wp_{d}{l}"].astype(bf))

    in_maps = []
    for i in range(NCORES):
        xb = batch[:, i * BC:(i + 1) * BC, :]           # [T, BC, F]
        xT_i = np.ascontiguousarray(
            xb.transpose(2, 0, 1).reshape(F, T * BC).astype(bf))  # [F, T*BC]
        in_maps.append({"xT": xT_i, **shared})
    return in_maps


def kernel(**inputs):
    from concourse.bass_utils import run_bass_kernel_spmd

    nc = _get_nc(T)
    in_maps = _make_in_maps(inputs)
    res = run_bass_kernel_spmd(nc, in_maps, core_ids=list(range(NCORES)))

    # assemble: out_ends [2(dir), 2(end), 128, NKH, BC] -> h [BC, 256]
    h = np.zeros((2, 2, B, PROJ), dtype=np.float32)    # [dir, end, B, PROJ]
    for i in range(NCORES):
        oe = res.results[i]["out_ends"].astype(np.float32)
        # h[b, kt*128 + p] = oe[.., p, kt, b]
        h[:, :, i * BC:(i + 1) * BC, :] = oe.transpose(0, 1, 4, 3, 2).reshape(
            2, 2, BC, PROJ)

    out0 = np.concatenate([h[0, 0], h[1, 0]], axis=1)   # t = 0
    outT = np.concatenate([h[0, 1], h[1, 1]], axis=1)   # t = T-1
    emb = (out0 + outT) / np.float32(2.0)
    ss = np.maximum(np.sum(emb * emb, axis=-1, keepdims=True), np.float32(1e-12))
    emb = emb / np.sqrt(ss)
    return emb.astype(np.float32)
